# revision 17
# baseline (speedup 1.0000x reference)
"""Trainium2 Bass kernel for nn_BatchDropTop (topk row masking).

Reference math: per sample b, act = sum_c x[b,c,:,:]^2  -> [H,W]; L2-normalize
over flattened (H,W) (a positive per-sample scale -- cannot change any
ordering, so it is skipped); row score = max_w act -> [H]; drop (zero) the
rh=8 rows with the largest score; out = x * row_mask.

Kernel strategy (pure data parallel, batch 64 -> 8 samples on each of 8
cores; per core, per sample):
  - DMA x[s] (2048x24x8 f32, 1.5 MB) into SBUF as [128p, 16k, 192hw]
    (partition p holds channels 16p..16p+15; contiguous 12KB per partition).
    Loads alternate between the sync and scalar HWDGE rings, stores ride
    gpsimd/scalar -- one ring tops out around 260 GB/s and loads sharing a
    ring with stores FIFO-block behind them.
  - ACT: square elementwise (two halves, so PE starts early).
  - Channel reduce split across engines: gpsimd pre-folds the last 4
    chunks (3 adds); PE runs 6 accumulating N=384 ones-matmuls over the
    rest plus one N=192 matmul for gpsimd's partial -> two partial sums
    [1, 2, 192] in PSUM, folded by one strided DVE reduce.  (fp32 PE
    matmul is dual-pass, 4 cyc/col, so PE paced the tail before the
    gpsimd offload.)
  - DVE: rowmax[1,24] = max over w; top8 = vector.max (8 largest, desc);
    mask[1,24,8] = (rowmax < top8[7]) as 1.0/0.0, with the compare input
    broadcast over w.  (Exactly the top-8 rows get 0; validated tie-free
    on the real inputs with 4.4e-5 min rel gap -- fp32 accumulation is
    required, bf16/tf32 noise would flip borderline rows.)
  - PE ones[1,128] K=1 matmul broadcasts the mask to [128,192] PSUM.
  - DVE: y = x * mask (mask AP broadcast over the chunk dim), DMA out in
    half-sample units.

Everything is read from HBM once and written once: 25.2 MB per core
~= 70 us at the ~358 GB/s per-core HBM roofline (716 GB/s per stack
shared by a core pair).  Measured: 74.4-74.9 us on 7/8 cores, 76 us
mean (NTFF), incl. ~7.7 us framework startup (entry barrier + const
loads + DMA first-byte) and ~8.6 us Tile exit barrier; ACT/PE/DVE/
gpsimd work hides under the DMA stream.
"""

import sys

import numpy as np

for _p in ("/opt/trn_rl_repo", "/root/.axon_site/_ro/trn_rl_repo"):
    if _p not in sys.path:
        sys.path.append(_p)

B, C, H, W = 64, 2048, 24, 8
N_CORES = 8
BS = B // N_CORES  # samples per core
P = 128            # SBUF partitions
KC = C // P        # channel chunks per sample
HW = H * W
RH = 8             # rows to drop == round(0.33 * 24)

_cache = {}


def _build_nc():
    from concourse import bacc, mybir, tile

    f32 = mybir.dt.float32
    nc = bacc.Bacc("TRN2", target_bir_lowering=False, debug=False,
                   num_devices=N_CORES)
    x_in = nc.dram_tensor("x", [BS, C, H, W], f32, kind="ExternalInput")
    y_out = nc.dram_tensor("out", [BS, C, H, W], f32, kind="ExternalOutput")

    with tile.TileContext(nc) as tc:
        with (
            tc.tile_pool(name="xp", bufs=BS - 1) as xp,
            tc.tile_pool(name="sq", bufs=3) as sqp,
            tc.tile_pool(name="yp", bufs=4) as yp,
            tc.tile_pool(name="const", bufs=1) as constp,
            tc.tile_pool(name="tmp", bufs=3) as tmpp,
            tc.tile_pool(name="small", bufs=BS) as smallp,
            tc.tile_pool(name="psA", bufs=3, space="PSUM") as psA,
            tc.tile_pool(name="psB", bufs=3, space="PSUM") as psB,
        ):
            ones_col = constp.tile([P, 1], f32)  # stationary K=128 reducer
            nc.vector.memset(ones_col[:], 1.0)
            ones_row = constp.tile([1, P], f32)  # stationary K=1 broadcaster
            nc.vector.memset(ones_row[:], 1.0)

            KH = KC // 2
            # Emit ALL loads first: with a full set of x buffers every load
            # enqueues immediately, and both HWDGE rings drain them densely.
            # Program order also guarantees the loads sit ahead of any store
            # on scalar's ring, so stores never FIFO-block a load.
            xts = []
            for s in range(BS):
                # Loads alternate whole samples between the two HWDGE
                # rings (fewest trigger dispatches on scalar, whose
                # sequencer also runs every square).  Sample 0 is split
                # across both rings to halve the pipeline-fill latency.
                ld_eng = nc.sync if s % 2 == 0 else nc.scalar
                xt = xp.tile([P, KC, HW], f32, tag="x")
                x_dram = x_in[s].rearrange("(p k) h w -> p k (h w)", p=P)
                if s == 0:
                    nc.sync.dma_start(out=xt[:, :KH, :], in_=x_dram[:, :KH, :])
                    nc.scalar.dma_start(out=xt[:, KH:, :], in_=x_dram[:, KH:, :])
                else:
                    ld_eng.dma_start(out=xt[:], in_=x_dram[:])
                xts.append(xt)

            # Store ring map -- HWDGE only.  SWDGE (gpsimd) stores are out:
            # Q7 descriptor emission contends with DVE's SBUF perf-mode
            # lock right when DVE is grinding back-to-back multiplies, and
            # on a core that falls slightly behind this collapses the whole
            # store stream (observed 15 us DMA stall).  Sync (no compute)
            # takes the early/mid stores; scalar takes the late ones, but
            # only at emission points AFTER every square it could block.
            store_eng = {0: nc.sync, 1: nc.sync, 2: nc.sync,
                         3: nc.sync, 4: nc.sync,
                         5: nc.scalar, 6: nc.scalar, 7: nc.scalar}
            deferred = []
            for s in range(BS):
                st_eng = store_eng[s]
                xt = xts[s]

                # Square in two halves so PE can start reducing half A
                # while ACT squares half B.
                xsq = sqp.tile([P, KC, HW], f32, tag="sq")
                nc.scalar.square(xsq[:, :KH, :], xt[:, :KH, :])
                nc.scalar.square(xsq[:, KH:, :], xt[:, KH:, :])

                # Channel reduction, split across engines: the fp32 PE
                # matmul runs dual-pass and is the late-phase pacer, so
                # the idle gpsimd pre-folds the last 6 chunks (10..15)
                # with 5 adds.  (Bigger batched gpsimd offloads measured
                # SLOWER: gpsimd is ~42-90 G/s with ~0.45us/op fixed cost,
                # and a deeper serial fold tree delays PE's final matmul
                # -- and the whole mask chain -- by several us a sample.)
                tA = tmpp.tile([P, HW], f32, tag="tA")
                nc.gpsimd.tensor_tensor(tA[:], xsq[:, KC - 6, :],
                                        xsq[:, KC - 5, :],
                                        op=mybir.AluOpType.add)
                tB = tmpp.tile([P, HW], f32, tag="tB")
                nc.gpsimd.tensor_tensor(tB[:], xsq[:, KC - 4, :],
                                        xsq[:, KC - 3, :],
                                        op=mybir.AluOpType.add)
                tC = tmpp.tile([P, HW], f32, tag="tC")
                nc.gpsimd.tensor_tensor(tC[:], xsq[:, KC - 2, :],
                                        xsq[:, KC - 1, :],
                                        op=mybir.AluOpType.add)
                tD = tmpp.tile([P, HW], f32, tag="tD")
                nc.gpsimd.tensor_tensor(tD[:], tA[:], tB[:],
                                        op=mybir.AluOpType.add)
                tE = tmpp.tile([P, HW], f32, tag="tE")
                nc.gpsimd.tensor_tensor(tE[:], tD[:], tC[:],
                                        op=mybir.AluOpType.add)

                # PE: 5 accumulating N=384 matmuls over chunks 0..9, plus
                # one N=192 matmul folding in gpsimd's partial.
                act2 = psA.tile([1, 2, HW], f32, tag="act")
                for j in range(KC // 2 - 3):
                    nc.tensor.matmul(
                        act2[:], ones_col[:], xsq[:, 2 * j:2 * j + 2, :],
                        start=(j == 0), stop=False,
                    )
                nc.tensor.matmul(act2[:, 0, :], ones_col[:], tE[:],
                                 start=False, stop=True)
                act = smallp.tile([1, HW], f32, tag="actsb")
                nc.vector.tensor_reduce(
                    act[:], act2[:].transpose([0, 2, 1]),
                    axis=mybir.AxisListType.X, op=mybir.AluOpType.add,
                )

                rowmax = smallp.tile([1, H], f32, tag="rowmax")
                nc.vector.tensor_reduce(
                    rowmax[:],
                    act[:].rearrange("p (h w) -> p h w", h=H),
                    axis=mybir.AxisListType.X,
                    op=mybir.AluOpType.max,
                )
                top8 = smallp.tile([1, RH], f32, tag="top8")
                nc.vector.max(top8[:], rowmax[:])
                # mask over (h, w) in one shot: compare rowmax (broadcast
                # over w) against the 8th-largest value.
                maskhw = smallp.tile([1, HW], f32, tag="maskhw")
                nc.vector.tensor_single_scalar(
                    maskhw[:].rearrange("p (h w) -> p h w", h=H),
                    rowmax[:].unsqueeze(2).broadcast_to([1, H, W]),
                    top8[0:1, RH - 1:RH],
                    mybir.AluOpType.is_lt,
                )

                mb = psB.tile([P, HW], f32, tag="mb")
                nc.tensor.matmul(mb[:], ones_row[:], maskhw[:],
                                 start=True, stop=True)

                # Multiply + store in half-sample units: finer pipelining
                # and a shorter end-of-kernel tail.  Scalar-ring store
                # triggers are NOT emitted here: they would sit ahead of a
                # later sample's squares in scalar's instruction stream and
                # their mask-wait would stall them.  They are deferred past
                # the last square (emitted after this loop).
                yt = yp.tile([P, KC, HW], f32, tag="y")
                y_dram = y_out[s].rearrange("(p k) h w -> p k (h w)", p=P)
                for half in range(2):
                    ksl = slice(half * KH, (half + 1) * KH)
                    nc.vector.tensor_tensor(
                        yt[:, ksl, :], xt[:, ksl, :],
                        mb[:].unsqueeze(1).broadcast_to([P, KH, HW]),
                        op=mybir.AluOpType.mult,
                    )
                    if st_eng is nc.sync:
                        st_eng.dma_start(out=y_dram[:, ksl, :],
                                         in_=yt[:, ksl, :])
                    else:
                        deferred.append((y_dram[:, ksl, :], yt[:, ksl, :]))

            for dst, src in deferred:
                nc.scalar.dma_start(out=dst, in_=src)

    nc.compile()
    return nc


def get_nc():
    if "nc" not in _cache:
        _cache["nc"] = _build_nc()
    return _cache["nc"]


def kernel(x):
    from concourse.bass_utils import run_bass_kernel_spmd

    x = np.ascontiguousarray(np.asarray(x, dtype=np.float32))
    assert x.shape == (B, C, H, W), x.shape
    nc = get_nc()
    in_maps = [{"x": x[i * BS:(i + 1) * BS]} for i in range(N_CORES)]
    res = run_bass_kernel_spmd(nc, in_maps, list(range(N_CORES)))
    return np.concatenate(
        [res.results[i]["out"] for i in range(N_CORES)], axis=0
    )



# revision 19
# speedup vs baseline: 1.0369x; 1.0369x over previous
"""Trainium2 Bass kernel for nn_BatchDropTop (topk row masking).

Reference math: per sample b, act = sum_c x[b,c,:,:]^2  -> [H,W]; L2-normalize
over flattened (H,W) (a positive per-sample scale -- cannot change any
ordering, so it is skipped); row score = max_w act -> [H]; drop (zero) the
rh=8 rows with the largest score; out = x * row_mask.

Kernel strategy (pure data parallel, batch 64 -> 8 samples on each of 8
cores; per core, per sample):
  - DMA x[s] (2048x24x8 f32, 1.5 MB) into SBUF as [128p, 16k, 192hw]
    (partition p holds channels 16p..16p+15; contiguous 12KB per partition).
    Loads alternate between the sync and scalar HWDGE rings, stores ride
    gpsimd/scalar -- one ring tops out around 260 GB/s and loads sharing a
    ring with stores FIFO-block behind them.
  - ACT: square elementwise (two halves, so PE starts early).
  - Channel reduce split across engines: gpsimd pre-folds the last 4
    chunks (3 adds); PE runs 6 accumulating N=384 ones-matmuls over the
    rest plus one N=192 matmul for gpsimd's partial -> two partial sums
    [1, 2, 192] in PSUM, folded by one strided DVE reduce.  (fp32 PE
    matmul is dual-pass, 4 cyc/col, so PE paced the tail before the
    gpsimd offload.)
  - DVE: rowmax[1,24] = max over w; top8 = vector.max (8 largest, desc);
    mask[1,24,8] = (rowmax < top8[7]) as 1.0/0.0, with the compare input
    broadcast over w.  (Exactly the top-8 rows get 0; validated tie-free
    on the real inputs with 4.4e-5 min rel gap -- fp32 accumulation is
    required, bf16/tf32 noise would flip borderline rows.)
  - PE ones[1,128] K=1 matmul broadcasts the mask to [128,192] PSUM.
  - DVE: y = x * mask (mask AP broadcast over the chunk dim), DMA out in
    half-sample units.

Everything is read from HBM once and written once: 25.2 MB per core
~= 70 us at the ~358 GB/s per-core HBM roofline (716 GB/s per stack
shared by a core pair).  Measured: 74.4-74.9 us on 7/8 cores, 76 us
mean (NTFF), incl. ~7.7 us framework startup (entry barrier + const
loads + DMA first-byte) and ~8.6 us Tile exit barrier; ACT/PE/DVE/
gpsimd work hides under the DMA stream.
"""

import sys

import numpy as np

for _p in ("/opt/trn_rl_repo", "/root/.axon_site/_ro/trn_rl_repo"):
    if _p not in sys.path:
        sys.path.append(_p)

B, C, H, W = 64, 2048, 24, 8
N_CORES = 8
BS = B // N_CORES  # samples per core
P = 128            # SBUF partitions
KC = C // P        # channel chunks per sample
HW = H * W
RH = 8             # rows to drop == round(0.33 * 24)

_cache = {}


def _build_nc():
    from concourse import bacc, mybir, tile

    f32 = mybir.dt.float32
    nc = bacc.Bacc("TRN2", target_bir_lowering=False, debug=False,
                   num_devices=N_CORES)
    x_in = nc.dram_tensor("x", [BS, C, H, W], f32, kind="ExternalInput")
    y_out = nc.dram_tensor("out", [BS, C, H, W], f32, kind="ExternalOutput")

    with tile.TileContext(nc) as tc:
        with (
            tc.tile_pool(name="xp", bufs=BS - 1) as xp,
            tc.tile_pool(name="sq", bufs=3) as sqp,
            tc.tile_pool(name="yp", bufs=4) as yp,
            tc.tile_pool(name="const", bufs=1) as constp,
            tc.tile_pool(name="tmp", bufs=3) as tmpp,
            tc.tile_pool(name="small", bufs=BS) as smallp,
            tc.tile_pool(name="psA", bufs=3, space="PSUM") as psA,
            tc.tile_pool(name="psB", bufs=3, space="PSUM") as psB,
        ):
            ones_col = constp.tile([P, 1], f32)  # stationary K=128 reducer
            nc.vector.memset(ones_col[:], 1.0)
            ones_row = constp.tile([1, P], f32)  # stationary K=1 broadcaster
            nc.vector.memset(ones_row[:], 1.0)

            KH = KC // 2
            # Emit ALL loads first: with a full set of x buffers every load
            # enqueues immediately, and both HWDGE rings drain them densely.
            # Program order also guarantees the loads sit ahead of any store
            # on scalar's ring, so stores never FIFO-block a load.
            # Chunk split between the rings: sync carries chunks 0..9
            # (PE's 5 accumulating matmuls), scalar carries 10..15 (the
            # shorter gpsimd fold path).  Splitting EVERY sample across
            # both rings makes each sample's data land ~2x sooner and
            # shrinks the blast radius of a single-ring stall: measured
            # straggler cores lose one HWDGE ring for ~8-10 us while the
            # other runs at full rate, and with alternating whole-sample
            # loads that delay hits 2 full samples' compute chains.
            KS = 10
            xts = []
            for s in range(BS):
                xt = xp.tile([P, KC, HW], f32, tag="x")
                x_dram = x_in[s].rearrange("(p k) h w -> p k (h w)", p=P)
                nc.sync.dma_start(out=xt[:, :KS, :], in_=x_dram[:, :KS, :])
                nc.scalar.dma_start(out=xt[:, KS:, :], in_=x_dram[:, KS:, :])
                xts.append(xt)

            # Store ring map -- HWDGE only.  SWDGE (gpsimd) stores are out:
            # Q7 descriptor emission contends with DVE's SBUF perf-mode
            # lock right when DVE is grinding back-to-back multiplies, and
            # on a core that falls slightly behind this collapses the whole
            # store stream (observed 15 us DMA stall).  Sync (no compute)
            # takes the early/mid stores; scalar takes the late ones, but
            # only at emission points AFTER every square it could block.
            store_eng = {0: nc.sync, 1: nc.sync, 2: nc.sync,
                         3: nc.scalar, 4: nc.scalar,
                         5: nc.scalar, 6: nc.scalar, 7: nc.scalar}
            deferred = []
            for s in range(BS):
                st_eng = store_eng[s]
                xt = xts[s]

                # Square in two pieces matching the load split, so each
                # square waits only on its own ring's half.
                xsq = sqp.tile([P, KC, HW], f32, tag="sq")
                nc.scalar.square(xsq[:, :KS, :], xt[:, :KS, :])
                nc.scalar.square(xsq[:, KS:, :], xt[:, KS:, :])

                # Channel reduction, split across engines: the fp32 PE
                # matmul runs dual-pass and is the late-phase pacer, so
                # the idle gpsimd pre-folds the last 6 chunks (10..15)
                # with 5 adds.  (Bigger batched gpsimd offloads measured
                # SLOWER: gpsimd is ~42-90 G/s with ~0.45us/op fixed cost,
                # and a deeper serial fold tree delays PE's final matmul
                # -- and the whole mask chain -- by several us a sample.)
                tA = tmpp.tile([P, HW], f32, tag="tA")
                nc.gpsimd.tensor_tensor(tA[:], xsq[:, KC - 6, :],
                                        xsq[:, KC - 5, :],
                                        op=mybir.AluOpType.add)
                tB = tmpp.tile([P, HW], f32, tag="tB")
                nc.gpsimd.tensor_tensor(tB[:], xsq[:, KC - 4, :],
                                        xsq[:, KC - 3, :],
                                        op=mybir.AluOpType.add)
                tC = tmpp.tile([P, HW], f32, tag="tC")
                nc.gpsimd.tensor_tensor(tC[:], xsq[:, KC - 2, :],
                                        xsq[:, KC - 1, :],
                                        op=mybir.AluOpType.add)
                tD = tmpp.tile([P, HW], f32, tag="tD")
                nc.gpsimd.tensor_tensor(tD[:], tA[:], tB[:],
                                        op=mybir.AluOpType.add)
                tE = tmpp.tile([P, HW], f32, tag="tE")
                nc.gpsimd.tensor_tensor(tE[:], tD[:], tC[:],
                                        op=mybir.AluOpType.add)

                # PE: 5 accumulating N=384 matmuls over chunks 0..9, plus
                # one N=192 matmul folding in gpsimd's partial.
                act2 = psA.tile([1, 2, HW], f32, tag="act")
                for j in range(KC // 2 - 3):
                    nc.tensor.matmul(
                        act2[:], ones_col[:], xsq[:, 2 * j:2 * j + 2, :],
                        start=(j == 0), stop=False,
                    )
                nc.tensor.matmul(act2[:, 0, :], ones_col[:], tE[:],
                                 start=False, stop=True)
                act = smallp.tile([1, HW], f32, tag="actsb")
                nc.vector.tensor_reduce(
                    act[:], act2[:].transpose([0, 2, 1]),
                    axis=mybir.AxisListType.X, op=mybir.AluOpType.add,
                )

                rowmax = smallp.tile([1, H], f32, tag="rowmax")
                nc.vector.tensor_reduce(
                    rowmax[:],
                    act[:].rearrange("p (h w) -> p h w", h=H),
                    axis=mybir.AxisListType.X,
                    op=mybir.AluOpType.max,
                )
                top8 = smallp.tile([1, RH], f32, tag="top8")
                nc.vector.max(top8[:], rowmax[:])
                # mask over (h, w) in one shot: compare rowmax (broadcast
                # over w) against the 8th-largest value.
                maskhw = smallp.tile([1, HW], f32, tag="maskhw")
                nc.vector.tensor_single_scalar(
                    maskhw[:].rearrange("p (h w) -> p h w", h=H),
                    rowmax[:].unsqueeze(2).broadcast_to([1, H, W]),
                    top8[0:1, RH - 1:RH],
                    mybir.AluOpType.is_lt,
                )

                mb = psB.tile([P, HW], f32, tag="mb")
                nc.tensor.matmul(mb[:], ones_row[:], maskhw[:],
                                 start=True, stop=True)

                # Multiply + store in half-sample units: finer pipelining
                # and a shorter end-of-kernel tail.  Scalar-ring store
                # triggers are NOT emitted here: they would sit ahead of a
                # later sample's squares in scalar's instruction stream and
                # their mask-wait would stall them.  They are deferred past
                # the last square (emitted after this loop).
                yt = yp.tile([P, KC, HW], f32, tag="y")
                y_dram = y_out[s].rearrange("(p k) h w -> p k (h w)", p=P)
                for half in range(2):
                    ksl = slice(half * KH, (half + 1) * KH)
                    nc.vector.tensor_tensor(
                        yt[:, ksl, :], xt[:, ksl, :],
                        mb[:].unsqueeze(1).broadcast_to([P, KH, HW]),
                        op=mybir.AluOpType.mult,
                    )
                    if st_eng is nc.sync:
                        st_eng.dma_start(out=y_dram[:, ksl, :],
                                         in_=yt[:, ksl, :])
                    else:
                        deferred.append((y_dram[:, ksl, :], yt[:, ksl, :]))

            for dst, src in deferred:
                nc.scalar.dma_start(out=dst, in_=src)

    nc.compile()
    return nc


def get_nc():
    if "nc" not in _cache:
        _cache["nc"] = _build_nc()
    return _cache["nc"]


def kernel(x):
    from concourse.bass_utils import run_bass_kernel_spmd

    x = np.ascontiguousarray(np.asarray(x, dtype=np.float32))
    assert x.shape == (B, C, H, W), x.shape
    nc = get_nc()
    in_maps = [{"x": x[i * BS:(i + 1) * BS]} for i in range(N_CORES)]
    res = run_bass_kernel_spmd(nc, in_maps, list(range(N_CORES)))
    return np.concatenate(
        [res.results[i]["out"] for i in range(N_CORES)], axis=0
    )

